# revision 8
# baseline (speedup 1.0000x reference)
"""Causal self-attention on 8 trn2 NeuronCores.

Full inputs in, full output out. Sharding: data-parallel over batch (B=4),
tensor-parallel over head groups (16 heads -> 2 groups of 8). core = 2*b + g.

Per-core math (T=2048, C=1024, 8 heads, D=64, group channels G=512):
  qT/kT: [64*(h%2)+d, h//2, t] layout so scores need no transposes
  scoresT[j,i] = sum_d kT[d,j] qT[d,i]   (q pre-scaled by 1/sqrt(D) on host)
  softmax without max-subtraction (scores ~ N(0,1) by construction; exp is
  exactly shift-invariant so this matches the reference softmax)
  causal mask via affine_select (fill 0 post-exp) on diagonal blocks only
  AV stationary is [v | ones] column halves (swapped by head parity): the
  PE writes attention output into the head's own 64-partition half and the
  softmax denominators, replicated 64x, into the other half -- M=128 (no
  M=65 penalty) and normalization is recip + mul with no partition gather.
  out oT[d,i] accumulated over key blocks jb; y = oT.T @ Wp per 128-row tile.

Software pipeline: chunk loop ic=0..3; QKV(ic+1) and proj(ic-1) psum groups
are emitted as PE filler between attention blocks of chunk ic so the scalar
engine's exp stream overlaps matmul work instead of serializing phase 2.

Host gather: y[b] = part[2b] + part[2b+1] + b_attn_v @ W_proj + b_proj
(q/k biases are added on-device; the v bias commutes through softmax).
"""

import numpy as np
from contextlib import ExitStack

import concourse.bass as bass
import concourse.tile as tile
from concourse import bacc, mybir
from concourse.bass_utils import run_bass_kernel_spmd

P = 128
B, T, C, H = 4, 2048, 1024, 16
D = 64
HG = 8          # heads per core
G = HG * D      # 512 head channels per core
CT = C // P     # 8 contraction tiles
TCH = T // 512  # 4 chunks of 512 tokens
NT = G // P     # 4 tiles of head channels

f32 = mybir.dt.float32
bf16 = mybir.dt.bfloat16
MM_DT = bf16
EXP_DT = bf16


def build_attention(nc: bass.Bass):
    xT = nc.dram_tensor("xT", [C, T], MM_DT, kind="ExternalInput")
    wq = nc.dram_tensor("wq", [C, G], MM_DT, kind="ExternalInput")
    wk = nc.dram_tensor("wk", [C, G], MM_DT, kind="ExternalInput")
    wv = nc.dram_tensor("wv", [C, G], MM_DT, kind="ExternalInput")
    wp = nc.dram_tensor("wp", [G, C], MM_DT, kind="ExternalInput")
    bq = nc.dram_tensor("bq", [P, NT], f32, kind="ExternalInput")
    bk = nc.dram_tensor("bk", [P, NT], f32, kind="ExternalInput")
    y = nc.dram_tensor("y", [T, C], f32, kind="ExternalOutput")

    with tile.TileContext(nc) as tc, ExitStack() as ctx:
        persist = ctx.enter_context(tc.tile_pool(name="persist", bufs=1))
        qT = persist.tile([P, NT, T], MM_DT)
        kT = persist.tile([P, NT, T], MM_DT)
        # AV stationary: per (token-tile, parity, head-pair): 128 cols =
        # [v | ones] with the v half at 64*parity. Ones prefilled once; v
        # copied per chunk. Layout [t, tt, h%2, h//2, col].
        v_aug = persist.tile([P, T // P, 2, HG // 2, P], MM_DT)
        oT = persist.tile([P, NT, T], MM_DT)
        x_sb = persist.tile([P, CT, T], MM_DT)
        wq_sb = persist.tile([P, CT, G], MM_DT)
        wk_sb = persist.tile([P, CT, G], MM_DT)
        wv_sb = persist.tile([P, CT, G], MM_DT)
        wp_sb = persist.tile([P, NT, C], MM_DT)
        bq_sb = persist.tile([P, NT], f32)
        bk_sb = persist.tile([P, NT], f32)

        # DMA order = first-use order: x(0), wq, wk, wv, x(1..3), wp, biases
        for ct in range(CT):
            nc.sync.dma_start(out=x_sb[:, ct, 0:512], in_=xT.ap()[P * ct:P * (ct + 1), 0:512])
        for w_sb, w_d in ((wq_sb, wq), (wk_sb, wk), (wv_sb, wv)):
            for ct in range(CT):
                nc.sync.dma_start(out=w_sb[:, ct, :], in_=w_d.ap()[P * ct:P * (ct + 1), :])
        for tch in range(1, TCH):
            for ct in range(CT):
                nc.sync.dma_start(
                    out=x_sb[:, ct, 512 * tch:512 * (tch + 1)],
                    in_=xT.ap()[P * ct:P * (ct + 1), 512 * tch:512 * (tch + 1)],
                )
        for nt in range(NT):
            nc.sync.dma_start(out=wp_sb[:, nt, :], in_=wp.ap()[P * nt:P * (nt + 1), :])
        nc.sync.dma_start(out=bq_sb, in_=bq.ap())
        nc.sync.dma_start(out=bk_sb, in_=bk.ap())

        # ones halves of v_aug: even heads cols 64:128, odd heads cols 0:64
        nc.vector.memset(v_aug[:, :, 0, :, D:P], 1.0)
        nc.vector.memset(v_aug[:, :, 1, :, 0:D], 1.0)

        ps_qkv = ctx.enter_context(tc.tile_pool(name="ps_qkv", bufs=2, space="PSUM"))
        ps_s = ctx.enter_context(tc.tile_pool(name="ps_s", bufs=2, space="PSUM"))
        ps_o = ctx.enter_context(tc.tile_pool(name="ps_o", bufs=2, space="PSUM"))
        epool = ctx.enter_context(tc.tile_pool(name="epool", bufs=6))
        rbpool = ctx.enter_context(tc.tile_pool(name="rbpool", bufs=4))
        ypool = ctx.enter_context(tc.tile_pool(name="ypool", bufs=4))

        def qkv_units(tch):
            units = []
            for nm, w_sb, b_sb, dstT in (("q", wq_sb, bq_sb, qT),
                                         ("k", wk_sb, bk_sb, kT)):
                for jt in range(NT):
                    def u(nm=nm, w_sb=w_sb, b_sb=b_sb, dstT=dstT, jt=jt, tch=tch):
                        ps = ps_qkv.tile([P, 512], f32, tag="qkv",
                                         name=f"pqk_{tch}_{jt}_{nm}")
                        for ct in range(CT):
                            nc.tensor.matmul(
                                ps, w_sb[:, ct, P * jt:P * (jt + 1)],
                                x_sb[:, ct, 512 * tch:512 * (tch + 1)],
                                start=(ct == 0), stop=(ct == CT - 1),
                            )
                        nc.vector.tensor_scalar_add(
                            out=dstT[:, jt, 512 * tch:512 * (tch + 1)],
                            in0=ps, scalar1=b_sb[:, jt:jt + 1],
                        )
                    units.append(u)
            for tt4 in range(4):
                def u(tt4=tt4, tch=tch):
                    tt = 4 * tch + tt4
                    ps = ps_qkv.tile([P, 512], f32, tag="qkv", name=f"pv_{tt}")
                    for ct in range(CT):
                        nc.tensor.matmul(
                            ps, x_sb[:, ct, P * tt:P * (tt + 1)], wv_sb[:, ct, :],
                            start=(ct == 0), stop=(ct == CT - 1),
                        )
                    psr = ps.rearrange("p (g2 par d) -> p g2 par d", g2=HG // 2, par=2)
                    nc.vector.tensor_copy(
                        out=v_aug[:, tt, 0, :, 0:D], in_=psr[:, :, 0, :])
                    nc.vector.tensor_copy(
                        out=v_aug[:, tt, 1, :, D:P], in_=psr[:, :, 1, :])
                units.append(u)
            return units

        def proj_units(tch):
            units = []
            for tt4 in range(4):
                for mc in range(C // 512):
                    def u(tt4=tt4, mc=mc, tch=tch):
                        tt = 4 * tch + tt4
                        y_ps = ps_qkv.tile([P, 512], f32, tag="qkv",
                                           name=f"y_{tt}_{mc}")
                        for nt in range(NT):
                            nc.tensor.matmul(
                                y_ps, oT[:, nt, P * tt:P * (tt + 1)],
                                wp_sb[:, nt, 512 * mc:512 * (mc + 1)],
                                start=(nt == 0), stop=(nt == NT - 1),
                            )
                        y_sb = ypool.tile([P, 512], f32, tag="ysb",
                                          name=f"ysb_{tt}_{mc}")
                        nc.vector.tensor_copy(out=y_sb, in_=y_ps)
                        nc.sync.dma_start(
                            out=y.ap()[P * tt:P * (tt + 1), 512 * mc:512 * (mc + 1)],
                            in_=y_sb,
                        )
                    units.append(u)
            return units

        for u in qkv_units(0):
            u()

        for ic in range(TCH):
            filler = []
            if ic + 1 < TCH:
                filler += qkv_units(ic + 1)
            if ic >= 1:
                filler += proj_units(ic - 1)
            n_blocks = 4 * (4 * ic + 4)
            # pop schedule: spread filler units evenly over attention blocks
            pace = n_blocks / max(1, len(filler)) if filler else 0.0
            popped = 0
            blk = 0
            for g2 in range(HG // 2):
                o_ps = {}
                for hh in range(2):
                    o_ps[hh] = ps_o.tile([P, 512], f32, tag="o",
                                         name=f"ops_{2 * g2 + hh}_{ic}")
                n_jb = 4 * ic + 4
                for jb in range(n_jb):
                    off = max(0, P * jb - 512 * ic)
                    w = 512 - off
                    s_big = ps_s.tile([P, 1024], f32, tag="s",
                                      name=f"sps_{g2}_{ic}_{jb}")
                    for hh in range(2):
                        band = 64 * hh
                        nc.tensor.matmul(
                            s_big[:, 512 * hh + off:512 * (hh + 1)],
                            kT[band:band + D, g2, P * jb:P * (jb + 1)],
                            qT[band:band + D, g2, 512 * ic + off:512 * (ic + 1)],
                            start=True, stop=True,
                        )
                    e_big = epool.tile([P, 2, 512], EXP_DT, tag="e",
                                       name=f"e_{g2}_{ic}_{jb}")
                    nc.scalar.activation(
                        out=e_big[:, :, off:],
                        in_=s_big.rearrange("p (h2 i) -> p h2 i", h2=2)[:, :, off:],
                        func=mybir.ActivationFunctionType.Exp,
                    )
                    if P * jb >= 512 * ic:  # diagonal triangle mask
                        for hh in range(2):
                            nc.gpsimd.affine_select(
                                out=e_big[:, hh, off:off + P],
                                in_=e_big[:, hh, off:off + P],
                                compare_op=mybir.AluOpType.is_ge,
                                fill=0.0, base=0, channel_multiplier=-1,
                                pattern=[[1, P]],
                            )
                    # filler between exp and AV hides the activation latency
                    blk += 1
                    while filler and popped < int(blk / pace + 1e-9):
                        filler.pop(0)()
                        popped += 1
                    for hh in range(2):
                        nc.tensor.matmul(
                            o_ps[hh][:, off:], v_aug[:, jb, hh, g2, :],
                            e_big[:, hh, off:],
                            start=(jb == 0), stop=(jb == n_jb - 1),
                        )
                # normalize pair: denominators sit replicated in the half
                # opposite each head's output half
                for hh in range(2):
                    h = 2 * g2 + hh
                    p_ = h % 2
                    o_half = slice(64 * p_, 64 * p_ + 64)
                    s_half = slice(64 * (1 - p_), 64 * (1 - p_) + 64)
                    rb = rbpool.tile([P, 512], f32, tag="rb", name=f"rb_{h}_{ic}")
                    nc.vector.reciprocal(rb[s_half, :], o_ps[hh][s_half, :])
                    nc.vector.tensor_mul(
                        out=oT[o_half, g2, 512 * ic:512 * (ic + 1)],
                        in0=o_ps[hh][o_half, :],
                        in1=rb[s_half, :],
                    )
            while filler:
                filler.pop(0)()

        for u in proj_units(TCH - 1):
            u()


_NC_CACHE = {}


def _get_nc():
    if "nc" not in _NC_CACHE:
        nc = bacc.Bacc("TRN2", debug=False, num_devices=8)
        build_attention(nc)
        nc.compile()
        _NC_CACHE["nc"] = nc
    return _NC_CACHE["nc"]


def kernel(x, W_attn, b_attn, W_proj, b_proj):
    x = np.asarray(x, dtype=np.float32)
    W_attn = np.asarray(W_attn, dtype=np.float32)
    b_attn = np.asarray(b_attn, dtype=np.float32)
    W_proj = np.asarray(W_proj, dtype=np.float32)
    b_proj = np.asarray(b_proj, dtype=np.float32)

    import ml_dtypes
    mm_np = ml_dtypes.bfloat16

    scale = 1.0 / np.sqrt(np.float32(D))
    in_maps = []
    for core in range(8):
        b, g = divmod(core, 2)
        cols = slice(G * g, G * (g + 1))
        bqs = (b_attn[0:C][cols] * scale).reshape(NT, 2, D).transpose(1, 2, 0).reshape(P, NT)
        bks = b_attn[C:2 * C][cols].reshape(NT, 2, D).transpose(1, 2, 0).reshape(P, NT)
        in_maps.append({
            "xT": np.ascontiguousarray(x[b].T).astype(mm_np),
            "wq": np.ascontiguousarray(W_attn[:, 0:C][:, cols] * scale).astype(mm_np),
            "wk": np.ascontiguousarray(W_attn[:, C:2 * C][:, cols]).astype(mm_np),
            "wv": np.ascontiguousarray(W_attn[:, 2 * C:3 * C][:, cols]).astype(mm_np),
            "wp": np.ascontiguousarray(W_proj[G * g:G * (g + 1), :]).astype(mm_np),
            "bq": np.ascontiguousarray(bqs),
            "bk": np.ascontiguousarray(bks),
        })

    res = run_bass_kernel_spmd(_get_nc(), in_maps, core_ids=list(range(8)))

    correction = b_attn[2 * C:3 * C] @ W_proj + b_proj  # [C]
    out = np.empty((B, T, C), dtype=np.float32)
    for b in range(B):
        out[b] = res.results[2 * b]["y"] + res.results[2 * b + 1]["y"] + correction
    return out


# revision 13
# speedup vs baseline: 1.4351x; 1.4351x over previous
"""Causal self-attention on 8 trn2 NeuronCores.

Full inputs in, full output out. Sharding: data-parallel over batch (B=4),
tensor-parallel over head groups (16 heads -> 2 groups of 8). core = 2*b + g.

Per-core math (T=2048, C=1024, 8 heads, D=64, group channels G=512):
  qT/kT: [64*(h%2)+d, h//2, t] layout so scores need no transposes
  scoresT[j,i] = sum_d kT[d,j] qT[d,i]   (q pre-scaled by 1/sqrt(D) on host)
  softmax without max-subtraction (scores ~ N(0,1) by construction; exp is
  exactly shift-invariant so this matches the reference softmax)
  causal mask via affine_select (fill 0 post-exp) on diagonal blocks only
  AV stationary is [v | ones] column halves (swapped by head parity): the
  PE writes attention output into the head's own 64-partition half and the
  softmax denominators, replicated 64x, into the other half -- M=128 (no
  M=65 penalty) and normalization is recip + mul with no partition gather.
  out oT[d,i] accumulated over key blocks jb; y = oT.T @ Wp per 128-row tile.

Software pipeline: chunk loop ic=0..3; QKV(ic+1) and proj(ic-1) psum groups
are emitted as PE filler between attention blocks of chunk ic so the scalar
engine's exp stream overlaps matmul work instead of serializing phase 2.

Host gather: y[b] = part[2b] + part[2b+1] + b_attn_v @ W_proj + b_proj
(q/k biases are added on-device; the v bias commutes through softmax).
"""

import numpy as np
from contextlib import ExitStack

import concourse.bass as bass
import concourse.tile as tile
from concourse import bacc, mybir
from concourse.bass_utils import run_bass_kernel_spmd

P = 128
B, T, C, H = 4, 2048, 1024, 16
D = 64
HG = 8          # heads per core
G = HG * D      # 512 head channels per core
CT = C // P     # 8 contraction tiles
TCH = T // 512  # 4 chunks of 512 tokens
NT = G // P     # 4 tiles of head channels

f32 = mybir.dt.float32
bf16 = mybir.dt.bfloat16
MM_DT = bf16
EXP_DT = bf16


def build_attention(nc: bass.Bass):
    xT = nc.dram_tensor("xT", [C, T], MM_DT, kind="ExternalInput")
    wq = nc.dram_tensor("wq", [C, G], MM_DT, kind="ExternalInput")
    wk = nc.dram_tensor("wk", [C, G], MM_DT, kind="ExternalInput")
    wv = nc.dram_tensor("wv", [C, G], MM_DT, kind="ExternalInput")
    wp = nc.dram_tensor("wp", [G, C], MM_DT, kind="ExternalInput")
    bq = nc.dram_tensor("bq", [P, NT], f32, kind="ExternalInput")
    bk = nc.dram_tensor("bk", [P, NT], f32, kind="ExternalInput")
    y = nc.dram_tensor("y", [T, C], f32, kind="ExternalOutput")

    with tile.TileContext(nc) as tc, ExitStack() as ctx:
        persist = ctx.enter_context(tc.tile_pool(name="persist", bufs=1))
        qT = persist.tile([P, NT, T], MM_DT)
        kT = persist.tile([P, NT, T], MM_DT)
        # AV stationary: per (token-tile, parity, head-pair): 128 cols =
        # [v | ones] with the v half at 64*parity. Ones prefilled once; v
        # copied per chunk. Layout [t, tt, h%2, h//2, col].
        v_aug = persist.tile([P, T // P, 2, HG // 2, P], MM_DT)
        oT = persist.tile([P, NT, T], MM_DT)
        x_sb = persist.tile([P, CT, T], MM_DT)
        wq_sb = persist.tile([P, CT, G], MM_DT)
        wk_sb = persist.tile([P, CT, G], MM_DT)
        wv_sb = persist.tile([P, CT, G], MM_DT)
        wp_sb = persist.tile([P, NT, C], MM_DT)
        bq_sb = persist.tile([P, NT], f32)
        bk_sb = persist.tile([P, NT], f32)

        # DMA order = first-use order: x(0), wq, wk, wv, x(1..3), wp, biases
        for ct in range(CT):
            nc.sync.dma_start(out=x_sb[:, ct, 0:512], in_=xT.ap()[P * ct:P * (ct + 1), 0:512])
        for w_sb, w_d in ((wq_sb, wq), (wk_sb, wk), (wv_sb, wv)):
            for ct in range(CT):
                nc.sync.dma_start(out=w_sb[:, ct, :], in_=w_d.ap()[P * ct:P * (ct + 1), :])
        nc.sync.dma_start(out=bq_sb, in_=bq.ap())
        nc.sync.dma_start(out=bk_sb, in_=bk.ap())
        for tch in range(1, TCH):
            for ct in range(CT):
                nc.sync.dma_start(
                    out=x_sb[:, ct, 512 * tch:512 * (tch + 1)],
                    in_=xT.ap()[P * ct:P * (ct + 1), 512 * tch:512 * (tch + 1)],
                )
        for nt in range(NT):
            nc.sync.dma_start(out=wp_sb[:, nt, :], in_=wp.ap()[P * nt:P * (nt + 1), :])

        # ones halves of v_aug: even heads cols 64:128, odd heads cols 0:64
        nc.vector.memset(v_aug[:, :, 0, :, D:P], 1.0)
        nc.vector.memset(v_aug[:, :, 1, :, 0:D], 1.0)

        ps_qkv = ctx.enter_context(tc.tile_pool(name="ps_qkv", bufs=2, space="PSUM"))
        ps_s = ctx.enter_context(tc.tile_pool(name="ps_s", bufs=2, space="PSUM"))
        ps_o = ctx.enter_context(tc.tile_pool(name="ps_o", bufs=2, space="PSUM"))
        epool = ctx.enter_context(tc.tile_pool(name="epool", bufs=6))
        rbpool = ctx.enter_context(tc.tile_pool(name="rbpool", bufs=6))
        ypool = ctx.enter_context(tc.tile_pool(name="ypool", bufs=4))

        def qkv_units(tch):
            units = []
            for nm, w_sb, b_sb, dstT in (("q", wq_sb, bq_sb, qT),
                                         ("k", wk_sb, bk_sb, kT)):
                for jt in range(NT):
                    def u(nm=nm, w_sb=w_sb, b_sb=b_sb, dstT=dstT, jt=jt, tch=tch):
                        ps = ps_qkv.tile([P, 512], f32, tag="qkv",
                                         name=f"pqk_{tch}_{jt}_{nm}")
                        for ct in range(CT):
                            nc.tensor.matmul(
                                ps, w_sb[:, ct, P * jt:P * (jt + 1)],
                                x_sb[:, ct, 512 * tch:512 * (tch + 1)],
                                start=(ct == 0), stop=(ct == CT - 1),
                            )
                        nc.vector.tensor_scalar_add(
                            out=dstT[:, jt, 512 * tch:512 * (tch + 1)],
                            in0=ps, scalar1=b_sb[:, jt:jt + 1],
                        )
                    units.append(u)
            for tt4 in range(4):
                def u(tt4=tt4, tch=tch):
                    tt = 4 * tch + tt4
                    ps = ps_qkv.tile([P, 512], f32, tag="qkv", name=f"pv_{tt}")
                    for ct in range(CT):
                        nc.tensor.matmul(
                            ps, x_sb[:, ct, P * tt:P * (tt + 1)], wv_sb[:, ct, :],
                            start=(ct == 0), stop=(ct == CT - 1),
                        )
                    psr = ps.rearrange("p (g2 par d) -> p g2 par d", g2=HG // 2, par=2)
                    nc.vector.tensor_copy(
                        out=v_aug[:, tt, 0, :, 0:D], in_=psr[:, :, 0, :])
                    nc.vector.tensor_copy(
                        out=v_aug[:, tt, 1, :, D:P], in_=psr[:, :, 1, :])
                units.append(u)
            return units

        def proj_units(tch):
            units = []
            for tt4 in range(4):
                for mc in range(C // 512):
                    def u(tt4=tt4, mc=mc, tch=tch):
                        tt = 4 * tch + tt4
                        y_ps = ps_qkv.tile([P, 512], f32, tag="qkv",
                                           name=f"y_{tt}_{mc}")
                        for nt in range(NT):
                            nc.tensor.matmul(
                                y_ps, oT[:, nt, P * tt:P * (tt + 1)],
                                wp_sb[:, nt, 512 * mc:512 * (mc + 1)],
                                start=(nt == 0), stop=(nt == NT - 1),
                            )
                        y_sb = ypool.tile([P, 512], f32, tag="ysb",
                                          name=f"ysb_{tt}_{mc}")
                        nc.vector.tensor_copy(out=y_sb, in_=y_ps)
                        nc.sync.dma_start(
                            out=y.ap()[P * tt:P * (tt + 1), 512 * mc:512 * (mc + 1)],
                            in_=y_sb,
                        )
                    units.append(u)
            return units

        for u in qkv_units(0):
            u()

        for ic in range(TCH):
            filler = []
            if ic + 1 < TCH:
                filler += qkv_units(ic + 1)
            if ic >= 1:
                filler += proj_units(ic - 1)
            n_blocks = 4 * (4 * ic + 4)
            # pop schedule: spread filler units evenly over attention blocks
            pace = n_blocks / max(1, len(filler)) if filler else 0.0
            popped = 0
            blk = 0
            for g2 in range(HG // 2):
                o_ps = {}
                for hh in range(2):
                    o_ps[hh] = ps_o.tile([P, 512], f32, tag="o",
                                         name=f"ops_{2 * g2 + hh}_{ic}")
                n_jb = 4 * ic + 4
                for jb in range(n_jb):
                    off = max(0, P * jb - 512 * ic)
                    w = 512 - off
                    s_big = ps_s.tile([P, 1024], f32, tag="s",
                                      name=f"sps_{g2}_{ic}_{jb}")
                    for hh in range(2):
                        band = 64 * hh
                        nc.tensor.matmul(
                            s_big[:, 512 * hh + off:512 * (hh + 1)],
                            kT[band:band + D, g2, P * jb:P * (jb + 1)],
                            qT[band:band + D, g2, 512 * ic + off:512 * (ic + 1)],
                            start=True, stop=True,
                        )
                    e_big = epool.tile([P, 2, 512], EXP_DT, tag="e",
                                       name=f"e_{g2}_{ic}_{jb}")
                    nc.scalar.activation(
                        out=e_big[:, :, off:],
                        in_=s_big.rearrange("p (h2 i) -> p h2 i", h2=2)[:, :, off:],
                        func=mybir.ActivationFunctionType.Exp,
                    )
                    if P * jb >= 512 * ic:  # diagonal triangle mask
                        for hh in range(2):
                            nc.gpsimd.affine_select(
                                out=e_big[:, hh, off:off + P],
                                in_=e_big[:, hh, off:off + P],
                                compare_op=mybir.AluOpType.is_ge,
                                fill=0.0, base=0, channel_multiplier=-1,
                                pattern=[[1, P]],
                            )
                    # filler between exp and AV hides the activation latency
                    blk += 1
                    while filler and popped < int(blk / pace + 1e-9):
                        filler.pop(0)()
                        popped += 1
                    for hh in range(2):
                        nc.tensor.matmul(
                            o_ps[hh][:, off:], v_aug[:, jb, hh, g2, :],
                            e_big[:, hh, off:],
                            start=(jb == 0), stop=(jb == n_jb - 1),
                        )
                # normalize pair: denominators sit replicated in the half
                # opposite each head's output half
                for hh in range(2):
                    h = 2 * g2 + hh
                    p_ = h % 2
                    o_half = slice(64 * p_, 64 * p_ + 64)
                    s_half = slice(64 * (1 - p_), 64 * (1 - p_) + 64)
                    # sums -> base-0 SBUF (approx recip needs SBUF input);
                    # mul mixes PSUM in0 with SBUF in1 so differing base
                    # partitions are legal (same-space inputs must match)
                    o_s = rbpool.tile([D, 512], f32, tag="ou", name=f"ou_{h}_{ic}")
                    nc.vector.tensor_copy(out=o_s, in_=o_ps[hh][s_half, :])
                    rb = rbpool.tile([D, 512], f32, tag="rb", name=f"rb_{h}_{ic}")
                    nc.vector.reciprocal_approx_fast(out=rb, in_=o_s)
                    nc.vector.tensor_mul(
                        out=oT[o_half, g2, 512 * ic:512 * (ic + 1)],
                        in0=o_ps[hh][o_half, :],
                        in1=rb,
                    )
            while filler:
                filler.pop(0)()

        for u in proj_units(TCH - 1):
            u()


_NC_CACHE = {}


def _get_nc():
    if "nc" not in _NC_CACHE:
        nc = bacc.Bacc("TRN2", debug=False, num_devices=8)
        build_attention(nc)
        nc.compile()
        _NC_CACHE["nc"] = nc
    return _NC_CACHE["nc"]


def kernel(x, W_attn, b_attn, W_proj, b_proj):
    x = np.asarray(x, dtype=np.float32)
    W_attn = np.asarray(W_attn, dtype=np.float32)
    b_attn = np.asarray(b_attn, dtype=np.float32)
    W_proj = np.asarray(W_proj, dtype=np.float32)
    b_proj = np.asarray(b_proj, dtype=np.float32)

    import ml_dtypes
    mm_np = ml_dtypes.bfloat16

    scale = 1.0 / np.sqrt(np.float32(D))
    in_maps = []
    for core in range(8):
        b, g = divmod(core, 2)
        cols = slice(G * g, G * (g + 1))
        bqs = (b_attn[0:C][cols] * scale).reshape(NT, 2, D).transpose(1, 2, 0).reshape(P, NT)
        bks = b_attn[C:2 * C][cols].reshape(NT, 2, D).transpose(1, 2, 0).reshape(P, NT)
        in_maps.append({
            "xT": np.ascontiguousarray(x[b].T).astype(mm_np),
            "wq": np.ascontiguousarray(W_attn[:, 0:C][:, cols] * scale).astype(mm_np),
            "wk": np.ascontiguousarray(W_attn[:, C:2 * C][:, cols]).astype(mm_np),
            "wv": np.ascontiguousarray(W_attn[:, 2 * C:3 * C][:, cols]).astype(mm_np),
            "wp": np.ascontiguousarray(W_proj[G * g:G * (g + 1), :]).astype(mm_np),
            "bq": np.ascontiguousarray(bqs),
            "bk": np.ascontiguousarray(bks),
        })

    res = run_bass_kernel_spmd(_get_nc(), in_maps, core_ids=list(range(8)))

    correction = b_attn[2 * C:3 * C] @ W_proj + b_proj  # [C]
    out = np.empty((B, T, C), dtype=np.float32)
    for b in range(B):
        out[b] = res.results[2 * b]["y"] + res.results[2 * b + 1]["y"] + correction
    return out


# revision 15
# speedup vs baseline: 1.4409x; 1.0041x over previous
"""Causal self-attention on 8 trn2 NeuronCores.

Full inputs in, full output out. Sharding: data-parallel over batch (B=4),
tensor-parallel over head groups (16 heads -> 2 groups of 8). core = 2*b + g.

Per-core math (T=2048, C=1024, 8 heads, D=64, group channels G=512):
  qT/kT: [64*(h%2)+d, h//2, t] layout so scores need no transposes
  scoresT[j,i] = sum_d kT[d,j] qT[d,i]   (q pre-scaled by 1/sqrt(D) on host)
  softmax without max-subtraction (scores ~ N(0,1) by construction; exp is
  exactly shift-invariant so this matches the reference softmax)
  causal mask via affine_select (fill 0 post-exp) on diagonal blocks only
  AV stationary is [v | ones] column halves (swapped by head parity): the
  PE writes attention output into the head's own 64-partition half and the
  softmax denominators, replicated 64x, into the other half -- M=128 (no
  M=65 penalty) and normalization is recip + mul with no partition gather.
  out oT[d,i] accumulated over key blocks jb; y = oT.T @ Wp per 128-row tile.

Software pipeline: chunk loop ic=0..3; QKV(ic+1) and proj(ic-1) psum groups
are emitted as PE filler between attention blocks of chunk ic so the scalar
engine's exp stream overlaps matmul work instead of serializing phase 2.

Host gather: y[b] = part[2b] + part[2b+1] + b_attn_v @ W_proj + b_proj
(q/k biases are added on-device; the v bias commutes through softmax).
"""

import numpy as np
from contextlib import ExitStack

import concourse.bass as bass
import concourse.tile as tile
from concourse import bacc, mybir
from concourse.bass_utils import run_bass_kernel_spmd

P = 128
B, T, C, H = 4, 2048, 1024, 16
D = 64
HG = 8          # heads per core
G = HG * D      # 512 head channels per core
CT = C // P     # 8 contraction tiles
TCH = T // 512  # 4 chunks of 512 tokens
NT = G // P     # 4 tiles of head channels

f32 = mybir.dt.float32
bf16 = mybir.dt.bfloat16
MM_DT = bf16
EXP_DT = bf16


def build_attention(nc: bass.Bass):
    xT = nc.dram_tensor("xT", [C, T], MM_DT, kind="ExternalInput")
    wq = nc.dram_tensor("wq", [C, G], MM_DT, kind="ExternalInput")
    wk = nc.dram_tensor("wk", [C, G], MM_DT, kind="ExternalInput")
    wv = nc.dram_tensor("wv", [C, G], MM_DT, kind="ExternalInput")
    wp = nc.dram_tensor("wp", [G, C], MM_DT, kind="ExternalInput")
    bq = nc.dram_tensor("bq", [P, NT], f32, kind="ExternalInput")
    bk = nc.dram_tensor("bk", [P, NT], f32, kind="ExternalInput")
    y = nc.dram_tensor("y", [T, C], f32, kind="ExternalOutput")

    with tile.TileContext(nc) as tc, ExitStack() as ctx:
        persist = ctx.enter_context(tc.tile_pool(name="persist", bufs=1))
        qT = persist.tile([P, NT, T], MM_DT)
        kT = persist.tile([P, NT, T], MM_DT)
        # AV stationary: per (token-tile, parity, head-pair): 128 cols =
        # [v | ones] with the v half at 64*parity. Ones prefilled once; v
        # copied per chunk. Layout [t, tt, h%2, h//2, col].
        v_aug = persist.tile([P, T // P, 2, HG // 2, P], MM_DT)
        oT = persist.tile([P, NT, T], MM_DT)
        x_sb = persist.tile([P, CT, T], MM_DT)
        wq_sb = persist.tile([P, CT, G], MM_DT)
        wk_sb = persist.tile([P, CT, G], MM_DT)
        wv_sb = persist.tile([P, CT, G], MM_DT)
        wp_sb = persist.tile([P, NT, C], MM_DT)
        bq_sb = persist.tile([P, NT], f32)
        bk_sb = persist.tile([P, NT], f32)

        # DMA order = first-use order: x(0), wq, wk, wv, x(1..3), wp, biases
        for ct in range(CT):
            nc.sync.dma_start(out=x_sb[:, ct, 0:512], in_=xT.ap()[P * ct:P * (ct + 1), 0:512])
        for w_sb, w_d in ((wq_sb, wq), (wk_sb, wk), (wv_sb, wv)):
            for ct in range(CT):
                nc.sync.dma_start(out=w_sb[:, ct, :], in_=w_d.ap()[P * ct:P * (ct + 1), :])
        nc.sync.dma_start(out=bq_sb, in_=bq.ap())
        nc.sync.dma_start(out=bk_sb, in_=bk.ap())
        for tch in range(1, TCH):
            for ct in range(CT):
                nc.sync.dma_start(
                    out=x_sb[:, ct, 512 * tch:512 * (tch + 1)],
                    in_=xT.ap()[P * ct:P * (ct + 1), 512 * tch:512 * (tch + 1)],
                )
        for nt in range(NT):
            nc.sync.dma_start(out=wp_sb[:, nt, :], in_=wp.ap()[P * nt:P * (nt + 1), :])

        # ones halves of v_aug: even heads cols 64:128, odd heads cols 0:64
        nc.vector.memset(v_aug[:, :, 0, :, D:P], 1.0)
        nc.vector.memset(v_aug[:, :, 1, :, 0:D], 1.0)

        # pre-warm the Exp activation table while the PE ramps on QKV(0)
        warm = persist.tile([1, 8], f32)
        nc.vector.memset(warm, 0.0)
        nc.scalar.activation(out=warm, in_=warm,
                             func=mybir.ActivationFunctionType.Exp)

        ps_qkv = ctx.enter_context(tc.tile_pool(name="ps_qkv", bufs=2, space="PSUM"))
        ps_s = ctx.enter_context(tc.tile_pool(name="ps_s", bufs=2, space="PSUM"))
        ps_o = ctx.enter_context(tc.tile_pool(name="ps_o", bufs=2, space="PSUM"))
        epool = ctx.enter_context(tc.tile_pool(name="epool", bufs=6))
        rbpool = ctx.enter_context(tc.tile_pool(name="rbpool", bufs=6))
        ypool = ctx.enter_context(tc.tile_pool(name="ypool", bufs=4))

        def qkv_units(tch):
            units = []
            for nm, w_sb, b_sb, dstT in (("q", wq_sb, bq_sb, qT),
                                         ("k", wk_sb, bk_sb, kT)):
                for jt in range(NT):
                    def u(nm=nm, w_sb=w_sb, b_sb=b_sb, dstT=dstT, jt=jt, tch=tch):
                        ps = ps_qkv.tile([P, 512], f32, tag="qkv",
                                         name=f"pqk_{tch}_{jt}_{nm}")
                        for ct in range(CT):
                            nc.tensor.matmul(
                                ps, w_sb[:, ct, P * jt:P * (jt + 1)],
                                x_sb[:, ct, 512 * tch:512 * (tch + 1)],
                                start=(ct == 0), stop=(ct == CT - 1),
                            )
                        nc.vector.tensor_scalar_add(
                            out=dstT[:, jt, 512 * tch:512 * (tch + 1)],
                            in0=ps, scalar1=b_sb[:, jt:jt + 1],
                        )
                    units.append(u)
            for tt4 in range(4):
                def u(tt4=tt4, tch=tch):
                    tt = 4 * tch + tt4
                    ps = ps_qkv.tile([P, 512], f32, tag="qkv", name=f"pv_{tt}")
                    for ct in range(CT):
                        nc.tensor.matmul(
                            ps, x_sb[:, ct, P * tt:P * (tt + 1)], wv_sb[:, ct, :],
                            start=(ct == 0), stop=(ct == CT - 1),
                        )
                    psr = ps.rearrange("p (g2 par d) -> p g2 par d", g2=HG // 2, par=2)
                    nc.vector.tensor_copy(
                        out=v_aug[:, tt, 0, :, 0:D], in_=psr[:, :, 0, :])
                    nc.vector.tensor_copy(
                        out=v_aug[:, tt, 1, :, D:P], in_=psr[:, :, 1, :])
                units.append(u)
            return units

        def proj_units(tch):
            units = []
            for tt4 in range(4):
                for mc in range(C // 512):
                    def u(tt4=tt4, mc=mc, tch=tch):
                        tt = 4 * tch + tt4
                        y_ps = ps_qkv.tile([P, 512], f32, tag="qkv",
                                           name=f"y_{tt}_{mc}")
                        for nt in range(NT):
                            nc.tensor.matmul(
                                y_ps, oT[:, nt, P * tt:P * (tt + 1)],
                                wp_sb[:, nt, 512 * mc:512 * (mc + 1)],
                                start=(nt == 0), stop=(nt == NT - 1),
                            )
                        y_sb = ypool.tile([P, 512], f32, tag="ysb",
                                          name=f"ysb_{tt}_{mc}")
                        nc.vector.tensor_copy(out=y_sb, in_=y_ps)
                        nc.sync.dma_start(
                            out=y.ap()[P * tt:P * (tt + 1), 512 * mc:512 * (mc + 1)],
                            in_=y_sb,
                        )
                    units.append(u)
            return units

        for u in qkv_units(0):
            u()

        for ic in range(TCH):
            filler = []
            if ic + 1 < TCH:
                filler += qkv_units(ic + 1)
            if ic >= 1:
                filler += proj_units(ic - 1)
            n_blocks = 4 * (4 * ic + 4)
            # pop schedule: spread filler units evenly over attention blocks
            pace = n_blocks / max(1, len(filler)) if filler else 0.0
            popped = 0
            blk = 0
            for g2 in range(HG // 2):
                o_ps = {}
                for hh in range(2):
                    o_ps[hh] = ps_o.tile([P, 512], f32, tag="o",
                                         name=f"ops_{2 * g2 + hh}_{ic}")
                n_jb = 4 * ic + 4

                def av(jb, off, e_big, n_jb=n_jb, g2=g2, o_ps=o_ps):
                    for hh in range(2):
                        nc.tensor.matmul(
                            o_ps[hh][:, off:], v_aug[:, jb, hh, g2, :],
                            e_big[:, hh, off:],
                            start=(jb == 0), stop=(jb == n_jb - 1),
                        )

                pend = None  # delay-1: AV(jb-1) after exp(jb) so its
                # semaphore wait is satisfied and LDWEIGHTS overlaps
                for jb in range(n_jb):
                    off = max(0, P * jb - 512 * ic)
                    s_big = ps_s.tile([P, 1024], f32, tag="s",
                                      name=f"sps_{g2}_{ic}_{jb}")
                    for hh in range(2):
                        band = 64 * hh
                        nc.tensor.matmul(
                            s_big[:, 512 * hh + off:512 * (hh + 1)],
                            kT[band:band + D, g2, P * jb:P * (jb + 1)],
                            qT[band:band + D, g2, 512 * ic + off:512 * (ic + 1)],
                            start=True, stop=True,
                        )
                    e_big = epool.tile([P, 2, 512], EXP_DT, tag="e",
                                       name=f"e_{g2}_{ic}_{jb}")
                    nc.scalar.activation(
                        out=e_big[:, :, off:],
                        in_=s_big.rearrange("p (h2 i) -> p h2 i", h2=2)[:, :, off:],
                        func=mybir.ActivationFunctionType.Exp,
                    )
                    if P * jb >= 512 * ic:  # diagonal triangle mask
                        for hh in range(2):
                            nc.gpsimd.affine_select(
                                out=e_big[:, hh, off:off + P],
                                in_=e_big[:, hh, off:off + P],
                                compare_op=mybir.AluOpType.is_ge,
                                fill=0.0, base=0, channel_multiplier=-1,
                                pattern=[[1, P]],
                            )
                    if pend is not None:
                        av(*pend)
                    blk += 1
                    while filler and popped < int(blk / pace + 1e-9):
                        filler.pop(0)()
                        popped += 1
                    pend = (jb, off, e_big)
                av(*pend)
                # normalize pair: denominators sit replicated in the half
                # opposite each head's output half
                for hh in range(2):
                    h = 2 * g2 + hh
                    p_ = h % 2
                    o_half = slice(64 * p_, 64 * p_ + 64)
                    s_half = slice(64 * (1 - p_), 64 * (1 - p_) + 64)
                    # sums -> base-0 SBUF (approx recip needs SBUF input);
                    # mul mixes PSUM in0 with SBUF in1 so differing base
                    # partitions are legal (same-space inputs must match)
                    o_s = rbpool.tile([D, 512], f32, tag="ou", name=f"ou_{h}_{ic}")
                    nc.vector.tensor_copy(out=o_s, in_=o_ps[hh][s_half, :])
                    rb = rbpool.tile([D, 512], f32, tag="rb", name=f"rb_{h}_{ic}")
                    nc.vector.reciprocal_approx_fast(out=rb, in_=o_s)
                    nc.vector.tensor_mul(
                        out=oT[o_half, g2, 512 * ic:512 * (ic + 1)],
                        in0=o_ps[hh][o_half, :],
                        in1=rb,
                    )
            while filler:
                filler.pop(0)()

        for u in proj_units(TCH - 1):
            u()


_NC_CACHE = {}


def _get_nc():
    if "nc" not in _NC_CACHE:
        nc = bacc.Bacc("TRN2", debug=False, num_devices=8)
        build_attention(nc)
        nc.compile()
        _NC_CACHE["nc"] = nc
    return _NC_CACHE["nc"]


def kernel(x, W_attn, b_attn, W_proj, b_proj):
    x = np.asarray(x, dtype=np.float32)
    W_attn = np.asarray(W_attn, dtype=np.float32)
    b_attn = np.asarray(b_attn, dtype=np.float32)
    W_proj = np.asarray(W_proj, dtype=np.float32)
    b_proj = np.asarray(b_proj, dtype=np.float32)

    import ml_dtypes
    mm_np = ml_dtypes.bfloat16

    scale = 1.0 / np.sqrt(np.float32(D))
    in_maps = []
    for core in range(8):
        b, g = divmod(core, 2)
        cols = slice(G * g, G * (g + 1))
        bqs = (b_attn[0:C][cols] * scale).reshape(NT, 2, D).transpose(1, 2, 0).reshape(P, NT)
        bks = b_attn[C:2 * C][cols].reshape(NT, 2, D).transpose(1, 2, 0).reshape(P, NT)
        in_maps.append({
            "xT": np.ascontiguousarray(x[b].T).astype(mm_np),
            "wq": np.ascontiguousarray(W_attn[:, 0:C][:, cols] * scale).astype(mm_np),
            "wk": np.ascontiguousarray(W_attn[:, C:2 * C][:, cols]).astype(mm_np),
            "wv": np.ascontiguousarray(W_attn[:, 2 * C:3 * C][:, cols]).astype(mm_np),
            "wp": np.ascontiguousarray(W_proj[G * g:G * (g + 1), :]).astype(mm_np),
            "bq": np.ascontiguousarray(bqs),
            "bk": np.ascontiguousarray(bks),
        })

    res = run_bass_kernel_spmd(_get_nc(), in_maps, core_ids=list(range(8)))

    correction = b_attn[2 * C:3 * C] @ W_proj + b_proj  # [C]
    out = np.empty((B, T, C), dtype=np.float32)
    for b in range(B):
        out[b] = res.results[2 * b]["y"] + res.results[2 * b + 1]["y"] + correction
    return out


# revision 16
# speedup vs baseline: 1.4974x; 1.0392x over previous
"""Causal self-attention on 8 trn2 NeuronCores.

Full inputs in, full output out. Sharding: data-parallel over batch (B=4),
tensor-parallel over head groups (16 heads -> 2 groups of 8). core = 2*b + g.

Per-core math (T=2048, C=1024, 8 heads, D=64, group channels G=512):
  qT/kT: [64*(h%2)+d, h//2, t] layout so scores need no transposes
  scoresT[j,i] = sum_d kT[d,j] qT[d,i]   (q pre-scaled by 1/sqrt(D) on host)
  softmax without max-subtraction (scores ~ N(0,1) by construction; exp is
  exactly shift-invariant so this matches the reference softmax)
  causal mask via affine_select (fill 0 post-exp) on diagonal blocks only
  AV stationary is [v | ones] column halves (swapped by head parity): the
  PE writes attention output into the head's own 64-partition half and the
  softmax denominators, replicated 64x, into the other half -- M=128 (no
  M=65 penalty) and normalization is recip + mul with no partition gather.
  out oT[d,i] accumulated over key blocks jb; y = oT.T @ Wp per 128-row tile.

Software pipeline: chunk loop ic=0..3; QKV(ic+1) and proj(ic-1) psum groups
are emitted as PE filler between attention blocks of chunk ic so the scalar
engine's exp stream overlaps matmul work instead of serializing phase 2.

Host gather: y[b] = part[2b] + part[2b+1] + b_attn_v @ W_proj + b_proj
(q/k biases are added on-device; the v bias commutes through softmax).
"""

import numpy as np
from contextlib import ExitStack

import concourse.bass as bass
import concourse.tile as tile
from concourse import bacc, mybir
from concourse.bass_utils import run_bass_kernel_spmd

P = 128
B, T, C, H = 4, 2048, 1024, 16
D = 64
HG = 8          # heads per core
G = HG * D      # 512 head channels per core
CT = C // P     # 8 contraction tiles
TCH = T // 512  # 4 chunks of 512 tokens
NT = G // P     # 4 tiles of head channels

f32 = mybir.dt.float32
bf16 = mybir.dt.bfloat16
MM_DT = bf16
EXP_DT = bf16


def build_attention(nc: bass.Bass):
    xT = nc.dram_tensor("xT", [C, T], MM_DT, kind="ExternalInput")
    wq = nc.dram_tensor("wq", [C, G], MM_DT, kind="ExternalInput")
    wk = nc.dram_tensor("wk", [C, G], MM_DT, kind="ExternalInput")
    wv = nc.dram_tensor("wv", [C, G], MM_DT, kind="ExternalInput")
    wp = nc.dram_tensor("wp", [G, C], MM_DT, kind="ExternalInput")
    bq = nc.dram_tensor("bq", [P, NT], f32, kind="ExternalInput")
    bk = nc.dram_tensor("bk", [P, NT], f32, kind="ExternalInput")
    y = nc.dram_tensor("y", [T, C], f32, kind="ExternalOutput")

    with tile.TileContext(nc) as tc, ExitStack() as ctx:
        persist = ctx.enter_context(tc.tile_pool(name="persist", bufs=1))
        qT = persist.tile([P, NT, T], MM_DT)
        kT = persist.tile([P, NT, T], MM_DT)
        # AV stationary: per (token-tile, parity, head-pair): 128 cols =
        # [v | ones] with the v half at 64*parity. Ones prefilled once; v
        # copied per chunk. Layout [t, tt, h%2, h//2, col].
        v_aug = persist.tile([P, T // P, 2, HG // 2, P], MM_DT)
        oT = persist.tile([P, NT, T], MM_DT)
        x_sb = persist.tile([P, CT, T], MM_DT)
        wq_sb = persist.tile([P, CT, G], MM_DT)
        wk_sb = persist.tile([P, CT, G], MM_DT)
        wv_sb = persist.tile([P, CT, G], MM_DT)
        wp_sb = persist.tile([P, NT, C], MM_DT)
        bq_sb = persist.tile([P, NT], f32)
        bk_sb = persist.tile([P, NT], f32)

        # Batched input DMAs (one trigger each; ~0.65us/trigger on Sync):
        # order = first-use order: x(0), wq, biases, wk, wv, x(1..3), wp
        xT_r = xT.ap().rearrange("(ct p) t -> p ct t", p=P)
        nc.sync.dma_start(out=x_sb[:, :, 0:512], in_=xT_r[:, :, 0:512])
        nc.sync.dma_start(out=wq_sb, in_=wq.ap().rearrange("(ct p) g -> p ct g", p=P))
        nc.sync.dma_start(out=bq_sb, in_=bq.ap())
        nc.sync.dma_start(out=bk_sb, in_=bk.ap())
        nc.sync.dma_start(out=wk_sb, in_=wk.ap().rearrange("(ct p) g -> p ct g", p=P))
        nc.sync.dma_start(out=wv_sb, in_=wv.ap().rearrange("(ct p) g -> p ct g", p=P))
        nc.sync.dma_start(out=x_sb[:, :, 512:T], in_=xT_r[:, :, 512:T])
        nc.sync.dma_start(out=wp_sb, in_=wp.ap().rearrange("(nt p) c -> p nt c", p=P))

        # ones halves of v_aug: even heads cols 64:128, odd heads cols 0:64
        nc.vector.memset(v_aug[:, :, 0, :, D:P], 1.0)
        nc.vector.memset(v_aug[:, :, 1, :, 0:D], 1.0)

        # pre-warm the Exp activation table while the PE ramps on QKV(0)
        warm = persist.tile([1, 8], f32)
        nc.vector.memset(warm, 0.0)
        nc.scalar.activation(out=warm, in_=warm,
                             func=mybir.ActivationFunctionType.Exp)

        ps_qkv = ctx.enter_context(tc.tile_pool(name="ps_qkv", bufs=2, space="PSUM"))
        ps_s = ctx.enter_context(tc.tile_pool(name="ps_s", bufs=2, space="PSUM"))
        ps_o = ctx.enter_context(tc.tile_pool(name="ps_o", bufs=2, space="PSUM"))
        epool = ctx.enter_context(tc.tile_pool(name="epool", bufs=6))
        rbpool = ctx.enter_context(tc.tile_pool(name="rbpool", bufs=6))
        ypool = ctx.enter_context(tc.tile_pool(name="ypool", bufs=4))

        def qkv_units(tch):
            units = []
            for nm, w_sb, b_sb, dstT in (("q", wq_sb, bq_sb, qT),
                                         ("k", wk_sb, bk_sb, kT)):
                for jt in range(NT):
                    def u(nm=nm, w_sb=w_sb, b_sb=b_sb, dstT=dstT, jt=jt, tch=tch):
                        ps = ps_qkv.tile([P, 512], f32, tag="qkv",
                                         name=f"pqk_{tch}_{jt}_{nm}")
                        for ct in range(CT):
                            nc.tensor.matmul(
                                ps, w_sb[:, ct, P * jt:P * (jt + 1)],
                                x_sb[:, ct, 512 * tch:512 * (tch + 1)],
                                start=(ct == 0), stop=(ct == CT - 1),
                            )
                        nc.vector.tensor_scalar_add(
                            out=dstT[:, jt, 512 * tch:512 * (tch + 1)],
                            in0=ps, scalar1=b_sb[:, jt:jt + 1],
                        )
                    units.append(u)
            for tt4 in range(4):
                def u(tt4=tt4, tch=tch):
                    tt = 4 * tch + tt4
                    ps = ps_qkv.tile([P, 512], f32, tag="qkv", name=f"pv_{tt}")
                    for ct in range(CT):
                        nc.tensor.matmul(
                            ps, x_sb[:, ct, P * tt:P * (tt + 1)], wv_sb[:, ct, :],
                            start=(ct == 0), stop=(ct == CT - 1),
                        )
                    psr = ps.rearrange("p (g2 par d) -> p g2 par d", g2=HG // 2, par=2)
                    nc.vector.tensor_copy(
                        out=v_aug[:, tt, 0, :, 0:D], in_=psr[:, :, 0, :])
                    nc.vector.tensor_copy(
                        out=v_aug[:, tt, 1, :, D:P], in_=psr[:, :, 1, :])
                units.append(u)
            return units

        def proj_units(tch):
            units = []
            for tt4 in range(4):
                for mc in range(C // 512):
                    def u(tt4=tt4, mc=mc, tch=tch):
                        tt = 4 * tch + tt4
                        y_ps = ps_qkv.tile([P, 512], f32, tag="qkv",
                                           name=f"y_{tt}_{mc}")
                        for nt in range(NT):
                            nc.tensor.matmul(
                                y_ps, oT[:, nt, P * tt:P * (tt + 1)],
                                wp_sb[:, nt, 512 * mc:512 * (mc + 1)],
                                start=(nt == 0), stop=(nt == NT - 1),
                            )
                        y_sb = ypool.tile([P, 512], f32, tag="ysb",
                                          name=f"ysb_{tt}_{mc}")
                        nc.vector.tensor_copy(out=y_sb, in_=y_ps)
                        nc.sync.dma_start(
                            out=y.ap()[P * tt:P * (tt + 1), 512 * mc:512 * (mc + 1)],
                            in_=y_sb,
                        )
                    units.append(u)
            return units

        for u in qkv_units(0):
            u()

        for ic in range(TCH):
            filler = []
            if ic + 1 < TCH:
                filler += qkv_units(ic + 1)
            if ic >= 1:
                filler += proj_units(ic - 1)
            n_blocks = 4 * (4 * ic + 4)
            # pop schedule: spread filler units evenly over attention blocks
            pace = n_blocks / max(1, len(filler)) if filler else 0.0
            popped = 0
            blk = 0
            for g2 in range(HG // 2):
                o_ps = {}
                for hh in range(2):
                    o_ps[hh] = ps_o.tile([P, 512], f32, tag="o",
                                         name=f"ops_{2 * g2 + hh}_{ic}")
                n_jb = 4 * ic + 4

                def av(jb, off, e_big, n_jb=n_jb, g2=g2, o_ps=o_ps):
                    for hh in range(2):
                        nc.tensor.matmul(
                            o_ps[hh][:, off:], v_aug[:, jb, hh, g2, :],
                            e_big[:, hh, off:],
                            start=(jb == 0), stop=(jb == n_jb - 1),
                        )

                pend = None  # delay-1: AV(jb-1) after exp(jb) so its
                # semaphore wait is satisfied and LDWEIGHTS overlaps
                for jb in range(n_jb):
                    off = max(0, P * jb - 512 * ic)
                    s_big = ps_s.tile([P, 1024], f32, tag="s",
                                      name=f"sps_{g2}_{ic}_{jb}")
                    for hh in range(2):
                        band = 64 * hh
                        nc.tensor.matmul(
                            s_big[:, 512 * hh + off:512 * (hh + 1)],
                            kT[band:band + D, g2, P * jb:P * (jb + 1)],
                            qT[band:band + D, g2, 512 * ic + off:512 * (ic + 1)],
                            start=True, stop=True,
                        )
                    e_big = epool.tile([P, 2, 512], EXP_DT, tag="e",
                                       name=f"e_{g2}_{ic}_{jb}")
                    nc.scalar.activation(
                        out=e_big[:, :, off:],
                        in_=s_big.rearrange("p (h2 i) -> p h2 i", h2=2)[:, :, off:],
                        func=mybir.ActivationFunctionType.Exp,
                    )
                    if P * jb >= 512 * ic:  # diagonal triangle mask
                        for hh in range(2):
                            nc.gpsimd.affine_select(
                                out=e_big[:, hh, off:off + P],
                                in_=e_big[:, hh, off:off + P],
                                compare_op=mybir.AluOpType.is_ge,
                                fill=0.0, base=0, channel_multiplier=-1,
                                pattern=[[1, P]],
                            )
                    if pend is not None:
                        av(*pend)
                    blk += 1
                    while filler and popped < int(blk / pace + 1e-9):
                        filler.pop(0)()
                        popped += 1
                    pend = (jb, off, e_big)
                av(*pend)
                # normalize pair: denominators sit replicated in the half
                # opposite each head's output half
                for hh in range(2):
                    h = 2 * g2 + hh
                    p_ = h % 2
                    o_half = slice(64 * p_, 64 * p_ + 64)
                    s_half = slice(64 * (1 - p_), 64 * (1 - p_) + 64)
                    # sums -> base-0 SBUF (approx recip needs SBUF input);
                    # mul mixes PSUM in0 with SBUF in1 so differing base
                    # partitions are legal (same-space inputs must match)
                    o_s = rbpool.tile([D, 512], f32, tag="ou", name=f"ou_{h}_{ic}")
                    nc.vector.tensor_copy(out=o_s, in_=o_ps[hh][s_half, :])
                    rb = rbpool.tile([D, 512], f32, tag="rb", name=f"rb_{h}_{ic}")
                    nc.vector.reciprocal_approx_fast(out=rb, in_=o_s)
                    nc.vector.tensor_mul(
                        out=oT[o_half, g2, 512 * ic:512 * (ic + 1)],
                        in0=o_ps[hh][o_half, :],
                        in1=rb,
                    )
            while filler:
                filler.pop(0)()

        for u in proj_units(TCH - 1):
            u()


_NC_CACHE = {}


def _get_nc():
    if "nc" not in _NC_CACHE:
        nc = bacc.Bacc("TRN2", debug=False, num_devices=8)
        build_attention(nc)
        nc.compile()
        _NC_CACHE["nc"] = nc
    return _NC_CACHE["nc"]


def kernel(x, W_attn, b_attn, W_proj, b_proj):
    x = np.asarray(x, dtype=np.float32)
    W_attn = np.asarray(W_attn, dtype=np.float32)
    b_attn = np.asarray(b_attn, dtype=np.float32)
    W_proj = np.asarray(W_proj, dtype=np.float32)
    b_proj = np.asarray(b_proj, dtype=np.float32)

    import ml_dtypes
    mm_np = ml_dtypes.bfloat16

    scale = 1.0 / np.sqrt(np.float32(D))
    in_maps = []
    for core in range(8):
        b, g = divmod(core, 2)
        cols = slice(G * g, G * (g + 1))
        bqs = (b_attn[0:C][cols] * scale).reshape(NT, 2, D).transpose(1, 2, 0).reshape(P, NT)
        bks = b_attn[C:2 * C][cols].reshape(NT, 2, D).transpose(1, 2, 0).reshape(P, NT)
        in_maps.append({
            "xT": np.ascontiguousarray(x[b].T).astype(mm_np),
            "wq": np.ascontiguousarray(W_attn[:, 0:C][:, cols] * scale).astype(mm_np),
            "wk": np.ascontiguousarray(W_attn[:, C:2 * C][:, cols]).astype(mm_np),
            "wv": np.ascontiguousarray(W_attn[:, 2 * C:3 * C][:, cols]).astype(mm_np),
            "wp": np.ascontiguousarray(W_proj[G * g:G * (g + 1), :]).astype(mm_np),
            "bq": np.ascontiguousarray(bqs),
            "bk": np.ascontiguousarray(bks),
        })

    res = run_bass_kernel_spmd(_get_nc(), in_maps, core_ids=list(range(8)))

    correction = b_attn[2 * C:3 * C] @ W_proj + b_proj  # [C]
    out = np.empty((B, T, C), dtype=np.float32)
    for b in range(B):
        out[b] = res.results[2 * b]["y"] + res.results[2 * b + 1]["y"] + correction
    return out


# revision 18
# speedup vs baseline: 1.5451x; 1.0319x over previous
"""Causal self-attention on 8 trn2 NeuronCores.

Full inputs in, full output out. Sharding: data-parallel over batch (B=4),
tensor-parallel over head groups (16 heads -> 2 groups of 8). core = 2*b + g.

Per-core math (T=2048, C=1024, 8 heads, D=64, group channels G=512):
  qT/kT: [64*(h%2)+d, h//2, t] layout so scores need no transposes
  scoresT[j,i] = sum_d kT[d,j] qT[d,i]   (q pre-scaled by 1/sqrt(D) on host)
  softmax without max-subtraction (scores ~ N(0,1) by construction; exp is
  exactly shift-invariant so this matches the reference softmax)
  causal mask via affine_select (fill 0 post-exp) on diagonal blocks only
  AV stationary is [v | ones] column halves (swapped by head parity): the
  PE writes attention output into the head's own 64-partition half and the
  softmax denominators, replicated 64x, into the other half -- M=128 (no
  M=65 penalty) and normalization is recip + mul with no partition gather.
  out oT[d,i] accumulated over key blocks jb; y = oT.T @ Wp per 128-row tile.

Software pipeline: chunk loop ic=0..3; QKV(ic+1) and proj(ic-1) psum groups
are emitted as PE filler between attention blocks of chunk ic so the scalar
engine's exp stream overlaps matmul work instead of serializing phase 2.

Host gather: y[b] = part[2b] + part[2b+1] + b_attn_v @ W_proj + b_proj
(q/k biases are added on-device; the v bias commutes through softmax).
"""

import numpy as np
from contextlib import ExitStack

import concourse.bass as bass
import concourse.tile as tile
from concourse import bacc, mybir
from concourse.bass_utils import run_bass_kernel_spmd

P = 128
B, T, C, H = 4, 2048, 1024, 16
D = 64
HG = 8          # heads per core
G = HG * D      # 512 head channels per core
CT = C // P     # 8 contraction tiles
TCH = T // 512  # 4 chunks of 512 tokens
NT = G // P     # 4 tiles of head channels

f32 = mybir.dt.float32
bf16 = mybir.dt.bfloat16
MM_DT = bf16
EXP_DT = bf16


def build_attention(nc: bass.Bass):
    xT = nc.dram_tensor("xT", [C, T], MM_DT, kind="ExternalInput")
    wq = nc.dram_tensor("wq", [C, G], MM_DT, kind="ExternalInput")
    wk = nc.dram_tensor("wk", [C, G], MM_DT, kind="ExternalInput")
    wv = nc.dram_tensor("wv", [C, G], MM_DT, kind="ExternalInput")
    wp = nc.dram_tensor("wp", [G, C], MM_DT, kind="ExternalInput")
    bq = nc.dram_tensor("bq", [P, NT], f32, kind="ExternalInput")
    bk = nc.dram_tensor("bk", [P, NT], f32, kind="ExternalInput")
    y = nc.dram_tensor("y", [T, C], f32, kind="ExternalOutput")

    with tile.TileContext(nc) as tc, ExitStack() as ctx:
        persist = ctx.enter_context(tc.tile_pool(name="persist", bufs=1))
        qT = persist.tile([P, NT, T], MM_DT)
        kT = persist.tile([P, NT, T], MM_DT)
        # AV stationary: per (token-tile, parity, head-pair): 128 cols =
        # [v | ones] with the v half at 64*parity. Ones prefilled once; v
        # copied per chunk. Layout [t, tt, h%2, h//2, col].
        v_aug = persist.tile([P, T // P, 2, HG // 2, P], MM_DT)
        oT = persist.tile([P, NT, T], MM_DT)
        x_sb = persist.tile([P, CT, T], MM_DT)
        wq_sb = persist.tile([P, CT, G], MM_DT)
        wk_sb = persist.tile([P, CT, G], MM_DT)
        wv_sb = persist.tile([P, CT, G], MM_DT)
        wp_sb = persist.tile([P, NT, C], MM_DT)
        bq_sb = persist.tile([P, NT], f32)
        bk_sb = persist.tile([P, NT], f32)

        # Batched input DMAs (one trigger each; ~0.65us/trigger on Sync):
        # order = first-use order: x(0), wq, biases, wk, wv, x(1..3), wp
        xT_r = xT.ap().rearrange("(ct p) t -> p ct t", p=P)
        wq_r = wq.ap().rearrange("(ct p) g -> p ct g", p=P)
        nc.sync.dma_start(out=x_sb[:, 0:4, 0:512], in_=xT_r[:, 0:4, 0:512])
        nc.sync.dma_start(out=wq_sb[:, 0:4, :], in_=wq_r[:, 0:4, :])
        nc.sync.dma_start(out=x_sb[:, 4:CT, 0:512], in_=xT_r[:, 4:CT, 0:512])
        nc.sync.dma_start(out=wq_sb[:, 4:CT, :], in_=wq_r[:, 4:CT, :])
        nc.sync.dma_start(out=bq_sb, in_=bq.ap())
        nc.sync.dma_start(out=bk_sb, in_=bk.ap())
        nc.sync.dma_start(out=wk_sb, in_=wk.ap().rearrange("(ct p) g -> p ct g", p=P))
        nc.sync.dma_start(out=wv_sb, in_=wv.ap().rearrange("(ct p) g -> p ct g", p=P))
        nc.sync.dma_start(out=x_sb[:, :, 512:T], in_=xT_r[:, :, 512:T])
        nc.sync.dma_start(out=wp_sb, in_=wp.ap().rearrange("(nt p) c -> p nt c", p=P))

        # ones halves of v_aug: even heads cols 64:128, odd heads cols 0:64
        nc.vector.memset(v_aug[:, :, 0, :, D:P], 1.0)
        nc.vector.memset(v_aug[:, :, 1, :, 0:D], 1.0)

        # pre-warm the Exp activation table while the PE ramps on QKV(0)
        warm = persist.tile([1, 8], f32)
        nc.vector.memset(warm, 0.0)
        nc.scalar.activation(out=warm, in_=warm,
                             func=mybir.ActivationFunctionType.Exp)

        ps_qkv = ctx.enter_context(tc.tile_pool(name="ps_qkv", bufs=2, space="PSUM"))
        ps_s = ctx.enter_context(tc.tile_pool(name="ps_s", bufs=2, space="PSUM"))
        ps_o = ctx.enter_context(tc.tile_pool(name="ps_o", bufs=2, space="PSUM"))
        epool = ctx.enter_context(tc.tile_pool(name="epool", bufs=6))
        rbpool = ctx.enter_context(tc.tile_pool(name="rbpool", bufs=6))
        ypool = ctx.enter_context(tc.tile_pool(name="ypool", bufs=4))

        def qkv_units(tch):
            units = []
            for nm, w_sb, b_sb, dstT in (("q", wq_sb, bq_sb, qT),
                                         ("k", wk_sb, bk_sb, kT)):
                for jt in range(NT):
                    def u(nm=nm, w_sb=w_sb, b_sb=b_sb, dstT=dstT, jt=jt, tch=tch):
                        ps = ps_qkv.tile([P, 512], f32, tag="qkv",
                                         name=f"pqk_{tch}_{jt}_{nm}")
                        for ct in range(CT):
                            nc.tensor.matmul(
                                ps, w_sb[:, ct, P * jt:P * (jt + 1)],
                                x_sb[:, ct, 512 * tch:512 * (tch + 1)],
                                start=(ct == 0), stop=(ct == CT - 1),
                            )
                        nc.vector.tensor_scalar_add(
                            out=dstT[:, jt, 512 * tch:512 * (tch + 1)],
                            in0=ps, scalar1=b_sb[:, jt:jt + 1],
                        )
                    units.append(u)
            for tt4 in range(4):
                def u(tt4=tt4, tch=tch):
                    tt = 4 * tch + tt4
                    ps = ps_qkv.tile([P, 512], f32, tag="qkv", name=f"pv_{tt}")
                    for ct in range(CT):
                        nc.tensor.matmul(
                            ps, x_sb[:, ct, P * tt:P * (tt + 1)], wv_sb[:, ct, :],
                            start=(ct == 0), stop=(ct == CT - 1),
                        )
                    psr = ps.rearrange("p (g2 par d) -> p g2 par d", g2=HG // 2, par=2)
                    nc.vector.tensor_copy(
                        out=v_aug[:, tt, 0, :, 0:D], in_=psr[:, :, 0, :])
                    nc.vector.tensor_copy(
                        out=v_aug[:, tt, 1, :, D:P], in_=psr[:, :, 1, :])
                units.append(u)
            return units

        def proj_units(tch):
            units = []
            for tt4 in range(4):
                for mc in range(C // 512):
                    def u(tt4=tt4, mc=mc, tch=tch):
                        tt = 4 * tch + tt4
                        y_ps = ps_qkv.tile([P, 512], f32, tag="qkv",
                                           name=f"y_{tt}_{mc}")
                        for nt in range(NT):
                            nc.tensor.matmul(
                                y_ps, oT[:, nt, P * tt:P * (tt + 1)],
                                wp_sb[:, nt, 512 * mc:512 * (mc + 1)],
                                start=(nt == 0), stop=(nt == NT - 1),
                            )
                        y_sb = ypool.tile([P, 512], f32, tag="ysb",
                                          name=f"ysb_{tt}_{mc}")
                        nc.vector.tensor_copy(out=y_sb, in_=y_ps)
                        nc.sync.dma_start(
                            out=y.ap()[P * tt:P * (tt + 1), 512 * mc:512 * (mc + 1)],
                            in_=y_sb,
                        )
                    units.append(u)
            return units

        for u in qkv_units(0):
            u()

        for ic in range(TCH):
            # filler: QKV(ic+1); proj lags two chunks so ic=3 (no QKV left,
            # ACT-heavy) still gets 16 units: proj(1)+proj(2)
            filler = []
            if ic + 1 < TCH:
                filler += qkv_units(ic + 1)
            if ic == 1:
                filler += proj_units(0)
            elif ic == 3:
                filler += proj_units(1) + proj_units(2)
            n_blocks = 4 * (4 * ic + 4)
            # pop schedule: spread filler units evenly over attention blocks
            pace = n_blocks / max(1, len(filler)) if filler else 0.0
            popped = 0
            blk = 0
            for g2 in range(HG // 2):
                o_ps = {}
                for hh in range(2):
                    o_ps[hh] = ps_o.tile([P, 512], f32, tag="o",
                                         name=f"ops_{2 * g2 + hh}_{ic}")
                n_jb = 4 * ic + 4

                def av(jb, off, e_big, n_jb=n_jb, g2=g2, o_ps=o_ps):
                    for hh in range(2):
                        nc.tensor.matmul(
                            o_ps[hh][:, off:], v_aug[:, jb, hh, g2, :],
                            e_big[:, hh, off:],
                            start=(jb == 0), stop=(jb == n_jb - 1),
                        )

                pend = None  # delay-1: AV(jb-1) after exp(jb) so its
                # semaphore wait is satisfied and LDWEIGHTS overlaps
                for jb in range(n_jb):
                    off = max(0, P * jb - 512 * ic)
                    s_big = ps_s.tile([P, 1024], f32, tag="s",
                                      name=f"sps_{g2}_{ic}_{jb}")
                    for hh in range(2):
                        band = 64 * hh
                        nc.tensor.matmul(
                            s_big[:, 512 * hh + off:512 * (hh + 1)],
                            kT[band:band + D, g2, P * jb:P * (jb + 1)],
                            qT[band:band + D, g2, 512 * ic + off:512 * (ic + 1)],
                            start=True, stop=True,
                        )
                    e_big = epool.tile([P, 2, 512], EXP_DT, tag="e",
                                       name=f"e_{g2}_{ic}_{jb}")
                    nc.scalar.activation(
                        out=e_big[:, :, off:],
                        in_=s_big.rearrange("p (h2 i) -> p h2 i", h2=2)[:, :, off:],
                        func=mybir.ActivationFunctionType.Exp,
                    )
                    if P * jb >= 512 * ic:  # diagonal triangle mask
                        for hh in range(2):
                            nc.gpsimd.affine_select(
                                out=e_big[:, hh, off:off + P],
                                in_=e_big[:, hh, off:off + P],
                                compare_op=mybir.AluOpType.is_ge,
                                fill=0.0, base=0, channel_multiplier=-1,
                                pattern=[[1, P]],
                            )
                    if pend is not None:
                        av(*pend)
                    blk += 1
                    while filler and popped < int(blk / pace + 1e-9):
                        filler.pop(0)()
                        popped += 1
                    pend = (jb, off, e_big)
                av(*pend)
                # normalize pair: denominators sit replicated in the half
                # opposite each head's output half
                for hh in range(2):
                    h = 2 * g2 + hh
                    p_ = h % 2
                    o_half = slice(64 * p_, 64 * p_ + 64)
                    s_half = slice(64 * (1 - p_), 64 * (1 - p_) + 64)
                    # sums -> base-0 SBUF (approx recip needs SBUF input);
                    # mul mixes PSUM in0 with SBUF in1 so differing base
                    # partitions are legal (same-space inputs must match)
                    o_s = rbpool.tile([D, 512], f32, tag="ou", name=f"ou_{h}_{ic}")
                    nc.vector.tensor_copy(out=o_s, in_=o_ps[hh][s_half, :])
                    rb = rbpool.tile([D, 512], f32, tag="rb", name=f"rb_{h}_{ic}")
                    nc.vector.reciprocal_approx_fast(out=rb, in_=o_s)
                    nc.vector.tensor_mul(
                        out=oT[o_half, g2, 512 * ic:512 * (ic + 1)],
                        in0=o_ps[hh][o_half, :],
                        in1=rb,
                    )
            while filler:
                filler.pop(0)()

        for u in proj_units(TCH - 1):
            u()


_NC_CACHE = {}


def _get_nc():
    if "nc" not in _NC_CACHE:
        nc = bacc.Bacc("TRN2", debug=False, num_devices=8)
        build_attention(nc)
        nc.compile()
        _NC_CACHE["nc"] = nc
    return _NC_CACHE["nc"]


def kernel(x, W_attn, b_attn, W_proj, b_proj):
    x = np.asarray(x, dtype=np.float32)
    W_attn = np.asarray(W_attn, dtype=np.float32)
    b_attn = np.asarray(b_attn, dtype=np.float32)
    W_proj = np.asarray(W_proj, dtype=np.float32)
    b_proj = np.asarray(b_proj, dtype=np.float32)

    import ml_dtypes
    mm_np = ml_dtypes.bfloat16

    scale = 1.0 / np.sqrt(np.float32(D))
    in_maps = []
    for core in range(8):
        b, g = divmod(core, 2)
        cols = slice(G * g, G * (g + 1))
        bqs = (b_attn[0:C][cols] * scale).reshape(NT, 2, D).transpose(1, 2, 0).reshape(P, NT)
        bks = b_attn[C:2 * C][cols].reshape(NT, 2, D).transpose(1, 2, 0).reshape(P, NT)
        in_maps.append({
            "xT": np.ascontiguousarray(x[b].T).astype(mm_np),
            "wq": np.ascontiguousarray(W_attn[:, 0:C][:, cols] * scale).astype(mm_np),
            "wk": np.ascontiguousarray(W_attn[:, C:2 * C][:, cols]).astype(mm_np),
            "wv": np.ascontiguousarray(W_attn[:, 2 * C:3 * C][:, cols]).astype(mm_np),
            "wp": np.ascontiguousarray(W_proj[G * g:G * (g + 1), :]).astype(mm_np),
            "bq": np.ascontiguousarray(bqs),
            "bk": np.ascontiguousarray(bks),
        })

    res = run_bass_kernel_spmd(_get_nc(), in_maps, core_ids=list(range(8)))

    correction = b_attn[2 * C:3 * C] @ W_proj + b_proj  # [C]
    out = np.empty((B, T, C), dtype=np.float32)
    for b in range(B):
        out[b] = res.results[2 * b]["y"] + res.results[2 * b + 1]["y"] + correction
    return out


# revision 21
# speedup vs baseline: 1.5729x; 1.0180x over previous
"""Causal self-attention on 8 trn2 NeuronCores.

Full inputs in, full output out. Sharding: data-parallel over batch (B=4),
tensor-parallel over head groups (16 heads -> 2 groups of 8). core = 2*b + g.

Per-core math (T=2048, C=1024, 8 heads, D=64, group channels G=512):
  qT/kT: [64*(h%2)+d, h//2, t] layout so scores need no transposes
  scoresT[j,i] = sum_d kT[d,j] qT[d,i]   (q pre-scaled by 1/sqrt(D) on host)
  softmax without max-subtraction (scores ~ N(0,1) by construction; exp is
  exactly shift-invariant so this matches the reference softmax)
  causal mask via affine_select (fill 0 post-exp) on diagonal blocks only
  AV stationary is [v | ones] column halves (swapped by head parity): the
  PE writes attention output into the head's own 64-partition half and the
  softmax denominators, replicated 64x, into the other half -- M=128 (no
  M=65 penalty) and normalization is recip + mul with no partition gather.
  out oT[d,i] accumulated over key blocks jb; y = oT.T @ Wp per 128-row tile.

Software pipeline: chunk loop ic=0..3; QKV(ic+1) and proj(ic-1) psum groups
are emitted as PE filler between attention blocks of chunk ic so the scalar
engine's exp stream overlaps matmul work instead of serializing phase 2.

Host gather: y[b] = part[2b] + part[2b+1] + b_attn_v @ W_proj + b_proj
(q/k biases are added on-device; the v bias commutes through softmax).
"""

import numpy as np
from contextlib import ExitStack

import concourse.bass as bass
import concourse.tile as tile
from concourse import bacc, mybir
from concourse.bass_utils import run_bass_kernel_spmd

P = 128
B, T, C, H = 4, 2048, 1024, 16
D = 64
HG = 8          # heads per core
G = HG * D      # 512 head channels per core
CT = C // P     # 8 contraction tiles
TCH = T // 512  # 4 chunks of 512 tokens
NT = G // P     # 4 tiles of head channels

f32 = mybir.dt.float32
bf16 = mybir.dt.bfloat16
MM_DT = bf16
EXP_DT = bf16


def build_attention(nc: bass.Bass):
    xT = nc.dram_tensor("xT", [C, T], MM_DT, kind="ExternalInput")
    wq = nc.dram_tensor("wq", [C, G], MM_DT, kind="ExternalInput")
    wk = nc.dram_tensor("wk", [C, G], MM_DT, kind="ExternalInput")
    wv = nc.dram_tensor("wv", [C, G], MM_DT, kind="ExternalInput")
    wp = nc.dram_tensor("wp", [G, C], MM_DT, kind="ExternalInput")
    bq = nc.dram_tensor("bq", [P, NT], f32, kind="ExternalInput")
    bk = nc.dram_tensor("bk", [P, NT], f32, kind="ExternalInput")
    y = nc.dram_tensor("y", [T, C], f32, kind="ExternalOutput")

    with tile.TileContext(nc) as tc, ExitStack() as ctx:
        persist = ctx.enter_context(tc.tile_pool(name="persist", bufs=1))
        qT = persist.tile([P, NT, T], MM_DT)
        kT = persist.tile([P, NT, T], MM_DT)
        # AV stationary: per (token-tile, parity, head-pair): 128 cols =
        # [v | ones] with the v half at 64*parity. Ones prefilled once; v
        # copied per chunk. Layout [t, tt, h%2, h//2, col].
        v_aug = persist.tile([P, T // P, 2, HG // 2, P], MM_DT)
        oT = persist.tile([P, NT, T], MM_DT)
        x_sb = persist.tile([P, CT, T], MM_DT)
        wq_sb = persist.tile([P, CT, G], MM_DT)
        wk_sb = persist.tile([P, CT, G], MM_DT)
        wv_sb = persist.tile([P, CT, G], MM_DT)
        wp_sb = persist.tile([P, NT, C], MM_DT)
        bq_sb = persist.tile([P, NT], f32)
        bk_sb = persist.tile([P, NT], f32)

        # Batched input DMAs (one trigger each; ~0.65us/trigger on Sync):
        # order = first-use order: x(0), wq, biases, wk, wv, x(1..3), wp
        xT_r = xT.ap().rearrange("(ct p) t -> p ct t", p=P)
        wq_r = wq.ap().rearrange("(ct p) g -> p ct g", p=P)
        for q in range(4):
            cs = slice(2 * q, 2 * q + 2)
            nc.sync.dma_start(out=x_sb[:, cs, 0:512], in_=xT_r[:, cs, 0:512])
            nc.sync.dma_start(out=wq_sb[:, cs, :], in_=wq_r[:, cs, :])
        nc.sync.dma_start(out=bq_sb, in_=bq.ap())
        nc.sync.dma_start(out=bk_sb, in_=bk.ap())
        nc.sync.dma_start(out=wk_sb, in_=wk.ap().rearrange("(ct p) g -> p ct g", p=P))
        nc.sync.dma_start(out=wv_sb, in_=wv.ap().rearrange("(ct p) g -> p ct g", p=P))
        nc.sync.dma_start(out=x_sb[:, :, 512:T], in_=xT_r[:, :, 512:T])
        nc.sync.dma_start(out=wp_sb, in_=wp.ap().rearrange("(nt p) c -> p nt c", p=P))

        # ones halves of v_aug: even heads cols 64:128, odd heads cols 0:64
        nc.vector.memset(v_aug[:, :, 0, :, D:P], 1.0)
        nc.vector.memset(v_aug[:, :, 1, :, 0:D], 1.0)

        # pre-warm the Exp activation table while the PE ramps on QKV(0)
        warm = persist.tile([1, 8], f32)
        nc.vector.memset(warm, 0.0)
        nc.scalar.activation(out=warm, in_=warm,
                             func=mybir.ActivationFunctionType.Exp)

        ps_qkv = ctx.enter_context(tc.tile_pool(name="ps_qkv", bufs=2, space="PSUM"))
        ps_s = ctx.enter_context(tc.tile_pool(name="ps_s", bufs=2, space="PSUM"))
        ps_o = ctx.enter_context(tc.tile_pool(name="ps_o", bufs=2, space="PSUM"))
        epool = ctx.enter_context(tc.tile_pool(name="epool", bufs=6))
        rbpool = ctx.enter_context(tc.tile_pool(name="rbpool", bufs=6))
        ypool = ctx.enter_context(tc.tile_pool(name="ypool", bufs=4))

        def qkv_units(tch):
            units = []
            for nm, w_sb, b_sb, dstT in (("q", wq_sb, bq_sb, qT),
                                         ("k", wk_sb, bk_sb, kT)):
                for jt in range(NT):
                    def u(nm=nm, w_sb=w_sb, b_sb=b_sb, dstT=dstT, jt=jt, tch=tch):
                        ps = ps_qkv.tile([P, 512], f32, tag="qkv",
                                         name=f"pqk_{tch}_{jt}_{nm}")
                        for ct in range(CT):
                            nc.tensor.matmul(
                                ps, w_sb[:, ct, P * jt:P * (jt + 1)],
                                x_sb[:, ct, 512 * tch:512 * (tch + 1)],
                                start=(ct == 0), stop=(ct == CT - 1),
                            )
                        nc.vector.tensor_scalar_add(
                            out=dstT[:, jt, 512 * tch:512 * (tch + 1)],
                            in0=ps, scalar1=b_sb[:, jt:jt + 1],
                        )
                    units.append(u)
            for tt4 in range(4):
                def u(tt4=tt4, tch=tch):
                    tt = 4 * tch + tt4
                    ps = ps_qkv.tile([P, 512], f32, tag="qkv", name=f"pv_{tt}")
                    for ct in range(CT):
                        nc.tensor.matmul(
                            ps, x_sb[:, ct, P * tt:P * (tt + 1)], wv_sb[:, ct, :],
                            start=(ct == 0), stop=(ct == CT - 1),
                        )
                    psr = ps.rearrange("p (g2 par d) -> p g2 par d", g2=HG // 2, par=2)
                    nc.vector.tensor_copy(
                        out=v_aug[:, tt, 0, :, 0:D], in_=psr[:, :, 0, :])
                    nc.vector.tensor_copy(
                        out=v_aug[:, tt, 1, :, D:P], in_=psr[:, :, 1, :])
                units.append(u)
            return units

        def proj_units(tch):
            units = []
            for tt4 in range(4):
                for mc in range(C // 512):
                    def u(tt4=tt4, mc=mc, tch=tch):
                        tt = 4 * tch + tt4
                        y_ps = ps_qkv.tile([P, 512], f32, tag="qkv",
                                           name=f"y_{tt}_{mc}")
                        for nt in range(NT):
                            nc.tensor.matmul(
                                y_ps, oT[:, nt, P * tt:P * (tt + 1)],
                                wp_sb[:, nt, 512 * mc:512 * (mc + 1)],
                                start=(nt == 0), stop=(nt == NT - 1),
                            )
                        y_sb = ypool.tile([P, 512], f32, tag="ysb",
                                          name=f"ysb_{tt}_{mc}")
                        nc.vector.tensor_copy(out=y_sb, in_=y_ps)
                        nc.sync.dma_start(
                            out=y.ap()[P * tt:P * (tt + 1), 512 * mc:512 * (mc + 1)],
                            in_=y_sb,
                        )
                    units.append(u)
            return units

        for u in qkv_units(0):
            u()

        for ic in range(TCH):
            # filler: QKV(ic+1) for ic<3; all deferred proj lands in ic=3
            # (no QKV left there and its exp stream is the local bottleneck)
            filler = []
            if ic + 1 < TCH:
                filler += qkv_units(ic + 1)
            if ic == 3:
                filler += proj_units(0) + proj_units(1) + proj_units(2)
            n_blocks = 4 * (4 * ic + 4)
            # pop schedule: spread filler units evenly over attention blocks
            pace = n_blocks / max(1, len(filler)) if filler else 0.0
            popped = 0
            blk = 0
            for g2 in range(HG // 2):
                o_ps = {}
                for hh in range(2):
                    o_ps[hh] = ps_o.tile([P, 512], f32, tag="o",
                                         name=f"ops_{2 * g2 + hh}_{ic}")
                n_jb = 4 * ic + 4

                def av(jb, off, e_big, n_jb=n_jb, g2=g2, o_ps=o_ps):
                    for hh in range(2):
                        nc.tensor.matmul(
                            o_ps[hh][:, off:], v_aug[:, jb, hh, g2, :],
                            e_big[:, hh, off:],
                            start=(jb == 0), stop=(jb == n_jb - 1),
                        )

                pend = None  # delay-1: AV(jb-1) after exp(jb) so its
                # semaphore wait is satisfied and LDWEIGHTS overlaps
                for jb in range(n_jb):
                    off = max(0, P * jb - 512 * ic)
                    s_big = ps_s.tile([P, 1024], f32, tag="s",
                                      name=f"sps_{g2}_{ic}_{jb}")
                    for hh in range(2):
                        band = 64 * hh
                        nc.tensor.matmul(
                            s_big[:, 512 * hh + off:512 * (hh + 1)],
                            kT[band:band + D, g2, P * jb:P * (jb + 1)],
                            qT[band:band + D, g2, 512 * ic + off:512 * (ic + 1)],
                            start=True, stop=True,
                        )
                    e_big = epool.tile([P, 2, 512], EXP_DT, tag="e",
                                       name=f"e_{g2}_{ic}_{jb}")
                    if off == 0:  # flat AP: fewer dims, less per-inst setup
                        nc.scalar.activation(
                            out=e_big.rearrange("p h2 i -> p (h2 i)"),
                            in_=s_big,
                            func=mybir.ActivationFunctionType.Exp,
                        )
                    else:
                        nc.scalar.activation(
                            out=e_big[:, :, off:],
                            in_=s_big.rearrange("p (h2 i) -> p h2 i", h2=2)[:, :, off:],
                            func=mybir.ActivationFunctionType.Exp,
                        )
                    if P * jb >= 512 * ic:  # diagonal triangle mask
                        for hh in range(2):
                            nc.gpsimd.affine_select(
                                out=e_big[:, hh, off:off + P],
                                in_=e_big[:, hh, off:off + P],
                                compare_op=mybir.AluOpType.is_ge,
                                fill=0.0, base=0, channel_multiplier=-1,
                                pattern=[[1, P]],
                            )
                    if pend is not None:
                        av(*pend)
                    blk += 1
                    while filler and popped < int(blk / pace + 1e-9):
                        filler.pop(0)()
                        popped += 1
                    pend = (jb, off, e_big)
                av(*pend)
                # normalize pair: denominators sit replicated in the half
                # opposite each head's output half
                for hh in range(2):
                    h = 2 * g2 + hh
                    p_ = h % 2
                    o_half = slice(64 * p_, 64 * p_ + 64)
                    s_half = slice(64 * (1 - p_), 64 * (1 - p_) + 64)
                    # sums -> base-0 SBUF (approx recip needs SBUF input);
                    # mul mixes PSUM in0 with SBUF in1 so differing base
                    # partitions are legal (same-space inputs must match)
                    o_s = rbpool.tile([D, 512], f32, tag="ou", name=f"ou_{h}_{ic}")
                    nc.vector.tensor_copy(out=o_s, in_=o_ps[hh][s_half, :])
                    rb = rbpool.tile([D, 512], f32, tag="rb", name=f"rb_{h}_{ic}")
                    nc.vector.reciprocal_approx_fast(out=rb, in_=o_s)
                    nc.vector.tensor_mul(
                        out=oT[o_half, g2, 512 * ic:512 * (ic + 1)],
                        in0=o_ps[hh][o_half, :],
                        in1=rb,
                    )
            while filler:
                filler.pop(0)()

        for u in proj_units(TCH - 1):
            u()


_NC_CACHE = {}


def _get_nc():
    if "nc" not in _NC_CACHE:
        nc = bacc.Bacc("TRN2", debug=False, num_devices=8)
        build_attention(nc)
        nc.compile()
        _NC_CACHE["nc"] = nc
    return _NC_CACHE["nc"]


def kernel(x, W_attn, b_attn, W_proj, b_proj):
    x = np.asarray(x, dtype=np.float32)
    W_attn = np.asarray(W_attn, dtype=np.float32)
    b_attn = np.asarray(b_attn, dtype=np.float32)
    W_proj = np.asarray(W_proj, dtype=np.float32)
    b_proj = np.asarray(b_proj, dtype=np.float32)

    import ml_dtypes
    mm_np = ml_dtypes.bfloat16

    scale = 1.0 / np.sqrt(np.float32(D))
    in_maps = []
    for core in range(8):
        b, g = divmod(core, 2)
        cols = slice(G * g, G * (g + 1))
        bqs = (b_attn[0:C][cols] * scale).reshape(NT, 2, D).transpose(1, 2, 0).reshape(P, NT)
        bks = b_attn[C:2 * C][cols].reshape(NT, 2, D).transpose(1, 2, 0).reshape(P, NT)
        in_maps.append({
            "xT": np.ascontiguousarray(x[b].T).astype(mm_np),
            "wq": np.ascontiguousarray(W_attn[:, 0:C][:, cols] * scale).astype(mm_np),
            "wk": np.ascontiguousarray(W_attn[:, C:2 * C][:, cols]).astype(mm_np),
            "wv": np.ascontiguousarray(W_attn[:, 2 * C:3 * C][:, cols]).astype(mm_np),
            "wp": np.ascontiguousarray(W_proj[G * g:G * (g + 1), :]).astype(mm_np),
            "bq": np.ascontiguousarray(bqs),
            "bk": np.ascontiguousarray(bks),
        })

    res = run_bass_kernel_spmd(_get_nc(), in_maps, core_ids=list(range(8)))

    correction = b_attn[2 * C:3 * C] @ W_proj + b_proj  # [C]
    out = np.empty((B, T, C), dtype=np.float32)
    for b in range(B):
        out[b] = res.results[2 * b]["y"] + res.results[2 * b + 1]["y"] + correction
    return out
